# revision 17
# baseline (speedup 1.0000x reference)
"""BertAttention (QKV proj + MHA + output proj + residual + LayerNorm) on 8 TRN2 NeuronCores.

Sharding: batch (4-way) x query-sequence-half (2-way) => 8 shards, no collectives.
Core c handles batch b=c//2, query half c%2. Each core computes K/V for its full
batch sequence (all heads) and Q/attention/output-proj/LayerNorm for its 1024
query rows. K/V projection work is duplicated across the 2 cores sharing a batch;
in exchange there is zero cross-core communication.

The host permutes each core's X rows so its query half comes first — attention is
permutation-invariant over keys as long as (K, V, mask) share the permutation, so
the program is identical across cores (pure SPMD) with no per-core indices.

Layouts (SBUF partition dim first):
  Xt, Kt:  [128, H/128, S]   transposed activations (feature on partitions), bf16
  Qt:      [128, H/128, SH]  transposed, bf16
  V:       [128, S/128, NH*65] natural ([tok, head-dim]) with a ones column per
           head at slot 64 — the PV matmul then yields sum(exp) as row 64 for free
  scores:  St[ktok, qtok] in PSUM; softmax sum over ktok (the partition dim) comes
           from the ones-column trick; max-subtraction safely skipped (|s| <~ 1)
  ctx:     [128, NH/2, SH]   transposed (head dim on partitions), bf16
  out:     natural [qtok, H] — residual add + LayerNorm along the free dim.
"""

from contextlib import ExitStack

import numpy as np

import bass_rust
import concourse.bass as bass
import concourse.mybir as mybir
from concourse.tile import TileContext
from concourse.bass_utils import run_bass_kernel_spmd

FP = mybir.dt.float32
BF = mybir.dt.bfloat16
AF = mybir.ActivationFunctionType
OP = mybir.AluOpType

N_CORES = 8
EPS = 1e-12

# The walrus build in this toolchain rejects instructions that carry more than
# one sync-wait command ("Too many sync wait commands", CoreV2/V3 setupSyncWait),
# while Tile freely attaches several semaphore waits to one instruction (and the
# TileContext exit drain aggregates one wait per logical processor). Hoist the
# excess waits onto standalone InstEventSemaphore carriers on the same engine,
# placed immediately before the instruction — engine streams are serial, so the
# gating semantics are identical.
_MAX_WAITS_PER_INST = 1


def _split_sync_waits(nc, cap=_MAX_WAITS_PER_INST):
    n_split = 0
    for fn in nc.m.functions:
        for bb in fn.blocks:
            insts = list(bb.instructions)
            out = []
            changed = False
            for ins in insts:
                si = ins.sync_info
                waits = list(si.on_wait) if (si is not None and si.on_wait) else []
                if len(waits) > cap:
                    head, tail = waits[: len(waits) - cap], waits[len(waits) - cap :]
                    for j, w in enumerate(head):
                        ev = mybir.InstEventSemaphore(
                            name=f"{ins.name}-sw{j}",
                            engine=ins.engine,
                            ins=[],
                            outs=[],
                            sync_info=bass_rust.SyncInfo(on_wait=[w], on_update=[]),
                        )
                        out.append(ev)
                        n_split += 1
                    si.on_wait = tail
                    changed = True
                out.append(ins)
            if changed:
                bb.instructions[:] = out
    return n_split


def _dram_row_bcast(handle, p, n):
    """AP reading DRAM vector [n] broadcast across p partitions."""
    return bass.AP(tensor=handle, offset=0, ap=[[0, p], [1, n]])


def _build(s, h, nh, sh, flags, split=True, stop_after=None):
    """Build the per-core Bass program. flags: which bias/affine inputs matter."""
    hd = h // nh
    assert hd == 64, "head packing assumes head_dim 64 (2 heads per 128 partitions)"
    kt_n = h // 128  # contraction tiles over hidden dim
    tt_n = s // 128  # key-token tiles
    qt_n = sh // 128  # query-token tiles
    qc = min(512, sh)  # matmul moving-dim chunk over query tokens
    scale = 1.0 / float(np.sqrt(hd))

    nc = bass.Bass(target_bir_lowering=False)
    x = nc.dram_tensor("x", [s, h], FP, kind="ExternalInput")
    mask = nc.dram_tensor("mask", [s], FP, kind="ExternalInput")
    w_dram = {
        n: nc.dram_tensor(n, [h, h], FP, kind="ExternalInput")
        for n in ("wq", "wk", "wv", "wo")
    }
    vec_dram = {
        n: nc.dram_tensor(n, [h], FP, kind="ExternalInput")
        for n in ("bq", "bk", "bv", "bo", "ln_gamma", "ln_beta")
        if flags[n]
    }
    out = nc.dram_tensor("out", [sh, h], FP, kind="ExternalOutput")

    with TileContext(nc) as tc, ExitStack() as st_all:
        persist = st_all.enter_context(tc.tile_pool(name="persist", bufs=1))
        dram = st_all.enter_context(tc.tile_pool(name="dram", bufs=1, space="DRAM"))
        st_mid = st_all.enter_context(ExitStack())
        # attention-phase SBUF pools allocated low in the stack so they do not
        # overlap the released weight/X zones (which would serialize phases)
        psb = st_mid.enter_context(tc.tile_pool(name="psb", bufs=2))
        rpool = st_mid.enter_context(tc.tile_pool(name="rpool", bufs=2))

        qt = persist.tile([128, kt_n, sh], BF)
        kt = persist.tile([128, kt_n, s], BF)
        vsb = persist.tile([128, tt_n, nh * 65], BF)
        ctx_t = persist.tile([128, nh // 2, sh], BF)
        wo_bf = persist.tile([128, kt_n, h], BF)
        mask_sb = persist.tile([128, tt_n], FP)
        eps_sb = persist.tile([128, 1], FP)
        xb_dram = dram.tile([s, h], BF)

        nc.vector.memset(eps_sb, EPS)
        nc.sync.dma_start(out=mask_sb, in_=mask[:].rearrange("(t p) -> p t", p=128))

        # bias columns for Qt/Kt evictions (partition = output feature in tile)
        bias_cols = {}
        for name in ("bq", "bk"):
            if flags[name]:
                col = persist.tile([128, kt_n], FP, name=f"{name}_col")
                nc.sync.dma_start(
                    out=col, in_=vec_dram[name][:].rearrange("(t p) -> p t", p=128)
                )
                bias_cols[name] = col
        # rows broadcast across partitions for V/out bias and LN affine
        bcast = {}
        for name in ("bv", "bo", "ln_gamma", "ln_beta"):
            if flags[name]:
                t = persist.tile([128, h], FP, name=f"{name}_bc")
                nc.sync.dma_start(out=t, in_=_dram_row_bcast(vec_dram[name], 128, h))
                bcast[name] = t

        # ones columns in V (slot 64 of each 65-wide head block)
        for m in range(tt_n):
            v_view = vsb[:, m, :].rearrange("p (a e) -> p a e", e=65)
            nc.vector.memset(v_view[:, :, 64:65], 1.0)

        projps = st_mid.enter_context(
            tc.tile_pool(name="projps", bufs=2, space="PSUM")
        )

        with ExitStack() as st_proj:
            xtpool = st_proj.enter_context(tc.tile_pool(name="xtpool", bufs=1))
            wbuf = st_proj.enter_context(tc.tile_pool(name="wbuf", bufs=2))
            stage = st_proj.enter_context(tc.tile_pool(name="stage", bufs=3))

            xt = xtpool.tile([128, kt_n, s], BF)

            hc = min(512, h)  # staging chunk (SBUF pressure)

            def load_weight(dname, w_bf=None):
                if w_bf is None:
                    w_bf = wbuf.tile([128, kt_n, h], BF, name=f"{dname}_bf", tag="w")
                for k in range(kt_n):
                    for c0 in range(0, h, hc):
                        stg = stage.tile([128, hc], FP, name="wstg", tag="stg")
                        nc.sync.dma_start(
                            out=stg, in_=w_dram[dname][k * 128 : (k + 1) * 128, c0 : c0 + hc]
                        )
                        nc.vector.tensor_copy(out=w_bf[:, k, c0 : c0 + hc], in_=stg)
                return w_bf

            # X: load f32, cast bf16, roundtrip through DRAM for the transpose
            for t in range(tt_n):
                for c0 in range(0, h, hc):
                    stg = stage.tile([128, hc], FP, name="xstg", tag="stg")
                    nc.sync.dma_start(out=stg, in_=x[t * 128 : (t + 1) * 128, c0 : c0 + hc])
                    xbt = stage.tile([128, hc], BF, name="xbt", tag="xbt")
                    nc.vector.tensor_copy(out=xbt, in_=stg)
                    nc.sync.dma_start(
                        out=xb_dram[t * 128 : (t + 1) * 128, c0 : c0 + hc], in_=xbt
                    )
            for k in range(kt_n):
                nc.sync.dma_start_transpose(
                    xt[:, k, :], xb_dram[:, k * 128 : (k + 1) * 128]
                )

            # Q^T / K^T projections: out[feature, token]
            for dname, dst, n_tok, bias_col in (
                ("wk", kt, s, bias_cols.get("bk")),
                ("wq", qt, sh, bias_cols.get("bq")),
            ):
                w_bf = load_weight(dname)
                for m in range(kt_n):
                    for n0 in range(0, n_tok, 512):
                        n1 = min(n0 + 512, n_tok)
                        ps = projps.tile([128, 512], FP, name="projp")
                        for k in range(kt_n):
                            nc.tensor.matmul(
                                ps[:, : n1 - n0],
                                w_bf[:, k, m * 128 : (m + 1) * 128],
                                xt[:, k, n0:n1],
                                start=(k == 0),
                                stop=(k == kt_n - 1),
                            )
                        if bias_col is not None:
                            nc.vector.tensor_scalar_add(
                                out=dst[:, m, n0:n1],
                                in0=ps[:, : n1 - n0],
                                scalar1=bias_col[:, m : m + 1],
                            )
                        else:
                            nc.vector.tensor_copy(
                                out=dst[:, m, n0:n1], in_=ps[:, : n1 - n0]
                            )

            # V projection: natural [token, feature] into 65-stride head blocks
            wv_bf = load_weight("wv")
            for m in range(tt_n):
                for n0 in range(0, h, 512):
                    ps = projps.tile([128, 512], FP, name="projp")
                    for k in range(kt_n):
                        nc.tensor.matmul(
                            ps,
                            xt[:, k, m * 128 : (m + 1) * 128],
                            wv_bf[:, k, n0 : n0 + 512],
                            start=(k == 0),
                            stop=(k == kt_n - 1),
                        )
                    dst = vsb[:, m, :].rearrange("p (a e) -> p a e", e=65)[
                        :, n0 // 64 : n0 // 64 + 8, 0:64
                    ]
                    src = ps.rearrange("p (a e) -> p a e", e=64)
                    if "bv" in bcast:
                        nc.vector.tensor_add(
                            out=dst,
                            in0=src,
                            in1=bcast["bv"][:, n0 : n0 + 512].rearrange(
                                "p (a e) -> p a e", e=64
                            ),
                        )
                    else:
                        nc.vector.tensor_copy(out=dst, in_=src)

            # Wo straight into its persistent buffer
            load_weight("wo", w_bf=wo_bf)
        # xtpool/wbuf/stage released here (SBUF); projps stays open so the
        # attention PSUM pools stack on top of it and phases can overlap.

        with (
            tc.tile_pool(name="stps", bufs=1, space="PSUM") as stps,
            tc.tile_pool(name="pvps", bufs=2, space="PSUM") as pvps,
        ):
            for hh in range(nh if stop_after != "proj" else 0):
                mt, po = hh // 2, 64 * (hh % 2)
                pv = pvps.tile([65, sh], FP, name="pvp")
                for m in range(tt_n):
                    stt = stps.tile([128, sh], FP, name="stp")
                    for n0 in range(0, sh, qc):
                        nc.tensor.matmul(
                            stt[:, n0 : n0 + qc],
                            kt[po : po + 64, mt, m * 128 : (m + 1) * 128],
                            qt[po : po + 64, mt, n0 : n0 + qc],
                            start=True,
                            stop=True,
                        )
                    p = psb.tile([128, sh], BF, name="pexp")
                    nc.scalar.activation(
                        p, stt, AF.Exp, bias=mask_sb[:, m : m + 1], scale=scale
                    )
                    for n0 in range(0, sh, qc):
                        nc.tensor.matmul(
                            pv[:, n0 : n0 + qc],
                            vsb[:, m, hh * 65 : (hh + 1) * 65],
                            p[:, n0 : n0 + qc],
                            start=(m == 0),
                            stop=(m == tt_n - 1),
                        )
                r = rpool.tile([1, sh], FP, name="recip")
                nc.vector.reciprocal(r, pv[64:65, :])
                # broadcast r across 64 partitions via a DRAM roundtrip (DMA
                # partition-broadcast needs a DRAM source on this toolchain)
                r_dram = dram.tile([sh], FP, name="rdram", tag="rdram", bufs=2)
                nc.sync.dma_start(out=r_dram, in_=r)
                rbc = rpool.tile([64, sh], FP, name="recipbc", bufs=1)
                nc.sync.dma_start(
                    out=rbc,
                    in_=bass.AP(tensor=r_dram.tensor, offset=r_dram.offset, ap=[[0, 64], [1, sh]]),
                )
                nc.vector.tensor_mul(
                    out=ctx_t[po : po + 64, mt, :], in0=pv[0:64, :], in1=rbc
                )

        st_mid.close()  # release projps (PSUM) + psb/rpool before output phase

        # ---- output projection + residual + LayerNorm (natural layout) ----
        with (
            tc.tile_pool(name="ops", bufs=4, space="PSUM") as ops,
            tc.tile_pool(name="osb", bufs=2) as osb,
            tc.tile_pool(name="lnp", bufs=2) as lnp,
        ):
            for m in range(qt_n if stop_after in (None, 'oproj') else 0):
                pss = []
                for n0 in range(0, h, 512):
                    ps = ops.tile([128, 512], FP, name="op")
                    # ctx_t tile mt holds heads 2mt / 2mt+1 on partitions
                    # 0-63 / 64-127, exactly matching Wo rows mt*128..(mt+1)*128,
                    # so one K=128 matmul contracts both heads at once.
                    for mt in range(nh // 2):
                        nc.tensor.matmul(
                            ps,
                            ctx_t[:, mt, m * 128 : (m + 1) * 128],
                            wo_bf[:, mt, n0 : n0 + 512],
                            start=(mt == 0),
                            stop=(mt == nh // 2 - 1),
                        )
                    pss.append((n0, ps))
                xres = osb.tile([128, h], FP, name="xres")
                nc.sync.dma_start(out=xres, in_=x[m * 128 : (m + 1) * 128, :])
                o = osb.tile([128, h], FP, name="osum")
                for n0, ps in pss:
                    nc.vector.tensor_add(
                        out=o[:, n0 : n0 + 512], in0=ps, in1=xres[:, n0 : n0 + 512]
                    )
                if "bo" in bcast:
                    nc.vector.tensor_add(out=o, in0=o, in1=bcast["bo"])
                if stop_after == "oproj":
                    nc.sync.dma_start(out=out[m * 128 : (m + 1) * 128, :], in_=o)
                    continue
                nsub = (h + 511) // 512
                stats = lnp.tile([128, nsub, 6], FP, name="stats")
                for i in range(nsub):
                    nc.vector.bn_stats(
                        out=stats[:, i, :], in_=o[:, i * 512 : (i + 1) * 512]
                    )
                mv = lnp.tile([128, 2], FP, name="mv")
                nc.vector.bn_aggr(out=mv, in_=stats)
                std = lnp.tile([128, 1], FP, name="std")
                nc.scalar.activation(std, mv[:, 1:2], AF.Sqrt, bias=eps_sb)
                inv = lnp.tile([128, 1], FP, name="inv")
                nc.vector.reciprocal(inv, std)
                y = osb.tile([128, h], FP, name="yout")
                nc.vector.tensor_scalar(
                    out=y,
                    in0=o,
                    scalar1=mv[:, 0:1],
                    scalar2=inv,
                    op0=OP.subtract,
                    op1=OP.mult,
                )
                if "ln_gamma" in bcast:
                    nc.vector.tensor_mul(out=y, in0=y, in1=bcast["ln_gamma"])
                if "ln_beta" in bcast:
                    nc.vector.tensor_add(out=y, in0=y, in1=bcast["ln_beta"])
                nc.sync.dma_start(out=out[m * 128 : (m + 1) * 128, :], in_=y)
            if stop_after not in (None, 'oproj'):
                for m in range(qt_n):
                    dbg = osb.tile([128, h], FP, name="dbg", tag="xres")
                    if stop_after == "proj":
                        nc.vector.tensor_copy(out=dbg, in_=kt[:, 0, 0:h])
                    else:
                        nc.vector.tensor_copy(out=dbg[0:64, :sh], in_=ctx_t[0:64, 0, :])
                        nc.vector.tensor_copy(out=dbg[64:128, :sh], in_=ctx_t[64:128, 0, :])
                    nc.sync.dma_start(out=out[m * 128 : (m + 1) * 128, :], in_=dbg)

    if split:
        _split_sync_waits(nc)
    return nc


_NC_CACHE = {}


def _get_nc(s, h, nh, sh, flags):
    key = (s, h, nh, sh, tuple(sorted(flags.items())))
    if key not in _NC_CACHE:
        _NC_CACHE[key] = _build(s, h, nh, sh, flags)
    return _NC_CACHE[key]


def _prepare(hidden_states, attention_mask, Wq, bq, Wk, bk, Wv, bv, Wo, bo, ln_gamma, ln_beta):
    hs = np.ascontiguousarray(np.asarray(hidden_states, dtype=np.float32))
    b_, s_, h_ = hs.shape
    nh_ = h_ // 64
    sh_ = s_ // 2
    am = np.asarray(attention_mask, dtype=np.float32).reshape(b_, s_)
    flags = {
        "bq": bool(np.any(np.asarray(bq))),
        "bk": bool(np.any(np.asarray(bk))),
        "bv": bool(np.any(np.asarray(bv))),
        "bo": bool(np.any(np.asarray(bo))),
        "ln_gamma": not bool(np.all(np.asarray(ln_gamma) == 1.0)),
        "ln_beta": bool(np.any(np.asarray(ln_beta))),
    }
    nc = _get_nc(s_, h_, nh_, sh_, flags)

    f32c = lambda a: np.ascontiguousarray(np.asarray(a, dtype=np.float32))
    shared = {"wq": f32c(Wq), "wk": f32c(Wk), "wv": f32c(Wv), "wo": f32c(Wo)}
    for name, arr in (
        ("bq", bq),
        ("bk", bk),
        ("bv", bv),
        ("bo", bo),
        ("ln_gamma", ln_gamma),
        ("ln_beta", ln_beta),
    ):
        if flags[name]:
            shared[name] = f32c(arr)

    in_maps = []
    for c in range(N_CORES):
        bb, half = c // 2, c % 2
        mine = slice(half * sh_, (half + 1) * sh_)
        other = slice((1 - half) * sh_, (2 - half) * sh_)
        xp = np.ascontiguousarray(np.concatenate([hs[bb, mine], hs[bb, other]], axis=0))
        mp = np.ascontiguousarray(np.concatenate([am[bb, mine], am[bb, other]]))
        in_maps.append({"x": xp, "mask": mp, **shared})
    return nc, in_maps, (b_, s_, h_, sh_)


def _assemble(results, shape):
    b_, s_, h_, sh_ = shape
    out = np.empty((b_, s_, h_), dtype=np.float32)
    for c in range(N_CORES):
        bb, half = c // 2, c % 2
        out[bb, half * sh_ : (half + 1) * sh_] = results[c]["out"]
    return out


def kernel(**inputs) -> np.ndarray:
    nc, in_maps, shape = _prepare(**inputs)
    res = run_bass_kernel_spmd(nc, in_maps, core_ids=list(range(N_CORES)))
    return _assemble(res.results, shape)


# revision 19
# speedup vs baseline: 1.4875x; 1.4875x over previous
"""BertAttention (QKV proj + MHA + output proj + residual + LayerNorm) on 8 TRN2 NeuronCores.

Sharding: batch (4-way) x query-sequence-half (2-way) => 8 shards, no collectives.
Core c handles batch b=c//2, query half c%2. Each core computes K/V for its full
batch sequence (all heads) and Q/attention/output-proj/LayerNorm for its 1024
query rows. K/V projection work is duplicated across the 2 cores sharing a batch;
in exchange there is zero cross-core communication.

The host permutes each core's X rows so its query half comes first — attention is
permutation-invariant over keys as long as (K, V, mask) share the permutation, so
the program is identical across cores (pure SPMD) with no per-core indices.

Layouts (SBUF partition dim first):
  Xt, Kt:  [128, H/128, S]   transposed activations (feature on partitions), bf16
  Qt:      [128, H/128, SH]  transposed, bf16
  V:       [128, S/128, NH*65] natural ([tok, head-dim]) with a ones column per
           head at slot 64 — the PV matmul then yields sum(exp) as row 64 for free
  scores:  St[ktok, qtok] in PSUM; softmax sum over ktok (the partition dim) comes
           from the ones-column trick; max-subtraction safely skipped (|s| <~ 1)
  ctx:     [128, NH/2, SH]   transposed (head dim on partitions), bf16
  out:     natural [qtok, H] — residual add + LayerNorm along the free dim.
"""

from contextlib import ExitStack

import numpy as np

import bass_rust
import concourse.bass as bass
import concourse.mybir as mybir
from concourse.tile import TileContext
from concourse.bass_utils import run_bass_kernel_spmd
from concourse.masks import make_identity

FP = mybir.dt.float32
BF = mybir.dt.bfloat16
AF = mybir.ActivationFunctionType
OP = mybir.AluOpType

N_CORES = 8
EPS = 1e-12

# The walrus build in this toolchain rejects instructions that carry more than
# one sync-wait command ("Too many sync wait commands", CoreV2/V3 setupSyncWait),
# while Tile freely attaches several semaphore waits to one instruction (and the
# TileContext exit drain aggregates one wait per logical processor). Hoist the
# excess waits onto standalone InstEventSemaphore carriers on the same engine,
# placed immediately before the instruction — engine streams are serial, so the
# gating semantics are identical.
_MAX_WAITS_PER_INST = 1


def _split_sync_waits(nc, cap=_MAX_WAITS_PER_INST):
    n_split = 0
    for fn in nc.m.functions:
        for bb in fn.blocks:
            insts = list(bb.instructions)
            out = []
            changed = False
            for ins in insts:
                si = ins.sync_info
                waits = list(si.on_wait) if (si is not None and si.on_wait) else []
                if len(waits) > cap:
                    head, tail = waits[: len(waits) - cap], waits[len(waits) - cap :]
                    for j, w in enumerate(head):
                        ev = mybir.InstEventSemaphore(
                            name=f"{ins.name}-sw{j}",
                            engine=ins.engine,
                            ins=[],
                            outs=[],
                            sync_info=bass_rust.SyncInfo(on_wait=[w], on_update=[]),
                        )
                        out.append(ev)
                        n_split += 1
                    si.on_wait = tail
                    changed = True
                out.append(ins)
            if changed:
                bb.instructions[:] = out
    return n_split


def _dram_row_bcast(handle, p, n):
    """AP reading DRAM vector [n] broadcast across p partitions."""
    return bass.AP(tensor=handle, offset=0, ap=[[0, p], [1, n]])


def _build(s, h, nh, sh, flags, split=True, stop_after=None):
    """Build the per-core Bass program. flags: which bias/affine inputs matter."""
    hd = h // nh
    assert hd == 64, "head packing assumes head_dim 64 (2 heads per 128 partitions)"
    kt_n = h // 128  # contraction tiles over hidden dim
    tt_n = s // 128  # key-token tiles
    qt_n = sh // 128  # query-token tiles
    qc = min(512, sh)  # matmul moving-dim chunk over query tokens
    scale = 1.0 / float(np.sqrt(hd))

    nc = bass.Bass(target_bir_lowering=False)
    x = nc.dram_tensor("x", [s, h], FP, kind="ExternalInput")
    mask = nc.dram_tensor("mask", [s], FP, kind="ExternalInput")
    w_dram = {
        n: nc.dram_tensor(n, [h, h], FP, kind="ExternalInput")
        for n in ("wq", "wk", "wv", "wo")
    }
    vec_dram = {
        n: nc.dram_tensor(n, [h], FP, kind="ExternalInput")
        for n in ("bq", "bk", "bv", "bo", "ln_gamma", "ln_beta")
        if flags[n]
    }
    out = nc.dram_tensor("out", [sh, h], FP, kind="ExternalOutput")

    with TileContext(nc) as tc, ExitStack() as st_all:
        persist = st_all.enter_context(tc.tile_pool(name="persist", bufs=1))
        dram = st_all.enter_context(tc.tile_pool(name="dram", bufs=1, space="DRAM"))
        st_mid = st_all.enter_context(ExitStack())
        # attention-phase SBUF pools allocated low in the stack so they do not
        # overlap the released weight/X zones (which would serialize phases)
        psb = st_mid.enter_context(tc.tile_pool(name="psb", bufs=2))
        rpool = st_mid.enter_context(tc.tile_pool(name="rpool", bufs=2))

        qt = persist.tile([128, kt_n, sh], BF)
        kt = persist.tile([128, kt_n, s], BF)
        vsb = persist.tile([128, tt_n, nh * 65], BF)
        ctx_t = persist.tile([128, nh // 2, sh], BF)
        wo_bf = persist.tile([128, kt_n, h], BF)
        mask_sb = persist.tile([128, tt_n], FP)
        eps_sb = persist.tile([128, 1], FP)

        nc.vector.memset(eps_sb, EPS)
        nc.sync.dma_start(out=mask_sb, in_=mask[:].rearrange("(t p) -> p t", p=128))

        # bias columns for Qt/Kt evictions (partition = output feature in tile)
        bias_cols = {}
        for name in ("bq", "bk"):
            if flags[name]:
                col = persist.tile([128, kt_n], FP, name=f"{name}_col")
                nc.sync.dma_start(
                    out=col, in_=vec_dram[name][:].rearrange("(t p) -> p t", p=128)
                )
                bias_cols[name] = col
        # rows broadcast across partitions for V/out bias and LN affine
        bcast = {}
        for name in ("bv", "bo", "ln_gamma", "ln_beta"):
            if flags[name]:
                t = persist.tile([128, h], FP, name=f"{name}_bc")
                nc.sync.dma_start(out=t, in_=_dram_row_bcast(vec_dram[name], 128, h))
                bcast[name] = t

        # ones columns in V (slot 64 of each 65-wide head block)
        for m in range(tt_n):
            v_view = vsb[:, m, :].rearrange("p (a e) -> p a e", e=65)
            nc.vector.memset(v_view[:, :, 64:65], 1.0)

        ident = persist.tile([128, 128], BF)
        make_identity(nc, ident)

        with ExitStack() as st_proj:
            xtpool = st_proj.enter_context(tc.tile_pool(name="xtpool", bufs=1))
            wbuf = st_proj.enter_context(tc.tile_pool(name="wbuf", bufs=2))
            stage = st_proj.enter_context(tc.tile_pool(name="stage", bufs=3))
            projps = st_proj.enter_context(
                tc.tile_pool(name="projps", bufs=2, space="PSUM")
            )
            tps = st_proj.enter_context(tc.tile_pool(name="tps", bufs=4, space="PSUM"))

            xt = xtpool.tile([128, kt_n, s], BF)

            hc = min(512, h)  # staging chunk (SBUF pressure)

            def load_weight(dname, w_bf=None):
                if w_bf is None:
                    w_bf = wbuf.tile([128, kt_n, h], BF, name=f"{dname}_bf", tag="w")
                for k in range(kt_n):
                    for c0 in range(0, h, hc):
                        stg = stage.tile([128, hc], FP, name="wstg", tag="stg")
                        nc.sync.dma_start(
                            out=stg, in_=w_dram[dname][k * 128 : (k + 1) * 128, c0 : c0 + hc]
                        )
                        nc.vector.tensor_copy(out=w_bf[:, k, c0 : c0 + hc], in_=stg)
                return w_bf

            # X: load f32, cast bf16, transpose on the (otherwise idle) TensorE —
            # this also pre-warms the PE clock gate before the projections.
            for t in range(tt_n):
                xbt = stage.tile([128, h], BF, name="xbt", tag="xbt")
                for c0 in range(0, h, hc):
                    stg = stage.tile([128, hc], FP, name="xstg", tag="stg")
                    nc.sync.dma_start(out=stg, in_=x[t * 128 : (t + 1) * 128, c0 : c0 + hc])
                    nc.vector.tensor_copy(out=xbt[:, c0 : c0 + hc], in_=stg)
                for k in range(kt_n):
                    tp = tps.tile([128, 128], BF, name="tp")
                    nc.tensor.transpose(tp, xbt[:, k * 128 : (k + 1) * 128], ident)
                    nc.scalar.activation(
                        out=xt[:, k, t * 128 : (t + 1) * 128], in_=tp, func=AF.Copy
                    )

            # Q^T / K^T projections: out[feature, token]
            for dname, dst, n_tok, bias_col in (
                ("wk", kt, s, bias_cols.get("bk")),
                ("wq", qt, sh, bias_cols.get("bq")),
            ):
                w_bf = load_weight(dname)
                for m in range(kt_n):
                    for n0 in range(0, n_tok, 512):
                        n1 = min(n0 + 512, n_tok)
                        ps = projps.tile([128, 512], FP, name="projp")
                        for k in range(kt_n):
                            nc.tensor.matmul(
                                ps[:, : n1 - n0],
                                w_bf[:, k, m * 128 : (m + 1) * 128],
                                xt[:, k, n0:n1],
                                start=(k == 0),
                                stop=(k == kt_n - 1),
                            )
                        if bias_col is not None:
                            nc.vector.tensor_scalar_add(
                                out=dst[:, m, n0:n1],
                                in0=ps[:, : n1 - n0],
                                scalar1=bias_col[:, m : m + 1],
                            )
                        else:
                            nc.vector.tensor_copy(
                                out=dst[:, m, n0:n1], in_=ps[:, : n1 - n0]
                            )

            # V projection: natural [token, feature] into 65-stride head blocks
            wv_bf = load_weight("wv")
            for m in range(tt_n):
                for n0 in range(0, h, 512):
                    ps = projps.tile([128, 512], FP, name="projp")
                    for k in range(kt_n):
                        nc.tensor.matmul(
                            ps,
                            xt[:, k, m * 128 : (m + 1) * 128],
                            wv_bf[:, k, n0 : n0 + 512],
                            start=(k == 0),
                            stop=(k == kt_n - 1),
                        )
                    dst = vsb[:, m, :].rearrange("p (a e) -> p a e", e=65)[
                        :, n0 // 64 : n0 // 64 + 8, 0:64
                    ]
                    src = ps.rearrange("p (a e) -> p a e", e=64)
                    if "bv" in bcast:
                        nc.vector.tensor_add(
                            out=dst,
                            in0=src,
                            in1=bcast["bv"][:, n0 : n0 + 512].rearrange(
                                "p (a e) -> p a e", e=64
                            ),
                        )
                    else:
                        nc.vector.tensor_copy(out=dst, in_=src)

            # Wo straight into its persistent buffer
            load_weight("wo", w_bf=wo_bf)
        # xtpool/wbuf/stage (SBUF) and projps/tps (PSUM) released here; the
        # attention pools below get the full 8 PSUM banks.

        with (
            tc.tile_pool(name="stps", bufs=2, space="PSUM") as stps,
            tc.tile_pool(name="pvps", bufs=2, space="PSUM") as pvps,
        ):
            for hh in range(nh if stop_after != "proj" else 0):
                mt, po = hh // 2, 64 * (hh % 2)
                pv = pvps.tile([65, sh], FP, name="pvp")
                for m in range(tt_n):
                    stt = stps.tile([128, sh], FP, name="stp")
                    for n0 in range(0, sh, qc):
                        nc.tensor.matmul(
                            stt[:, n0 : n0 + qc],
                            kt[po : po + 64, mt, m * 128 : (m + 1) * 128],
                            qt[po : po + 64, mt, n0 : n0 + qc],
                            start=True,
                            stop=True,
                        )
                    p = psb.tile([128, sh], BF, name="pexp")
                    nc.scalar.activation(
                        p, stt, AF.Exp, bias=mask_sb[:, m : m + 1], scale=scale
                    )
                    for n0 in range(0, sh, qc):
                        nc.tensor.matmul(
                            pv[:, n0 : n0 + qc],
                            vsb[:, m, hh * 65 : (hh + 1) * 65],
                            p[:, n0 : n0 + qc],
                            start=(m == 0),
                            stop=(m == tt_n - 1),
                        )
                r = rpool.tile([1, sh], FP, name="recip")
                nc.vector.reciprocal(r, pv[64:65, :])
                # broadcast r across 64 partitions via a DRAM roundtrip (DMA
                # partition-broadcast needs a DRAM source on this toolchain)
                r_dram = dram.tile([sh], FP, name="rdram", tag="rdram", bufs=2)
                nc.sync.dma_start(out=r_dram, in_=r)
                rbc = rpool.tile([64, sh], FP, name="recipbc", bufs=1)
                nc.sync.dma_start(
                    out=rbc,
                    in_=bass.AP(tensor=r_dram.tensor, offset=r_dram.offset, ap=[[0, 64], [1, sh]]),
                )
                nc.vector.tensor_mul(
                    out=ctx_t[po : po + 64, mt, :], in0=pv[0:64, :], in1=rbc
                )

        st_mid.close()  # release projps (PSUM) + psb/rpool before output phase

        # ---- output projection + residual + LayerNorm (natural layout) ----
        with (
            tc.tile_pool(name="ops", bufs=4, space="PSUM") as ops,
            tc.tile_pool(name="osb", bufs=2) as osb,
            tc.tile_pool(name="lnp", bufs=2) as lnp,
        ):
            for m in range(qt_n if stop_after in (None, 'oproj') else 0):
                pss = []
                for n0 in range(0, h, 512):
                    ps = ops.tile([128, 512], FP, name="op")
                    # ctx_t tile mt holds heads 2mt / 2mt+1 on partitions
                    # 0-63 / 64-127, exactly matching Wo rows mt*128..(mt+1)*128,
                    # so one K=128 matmul contracts both heads at once.
                    for mt in range(nh // 2):
                        nc.tensor.matmul(
                            ps,
                            ctx_t[:, mt, m * 128 : (m + 1) * 128],
                            wo_bf[:, mt, n0 : n0 + 512],
                            start=(mt == 0),
                            stop=(mt == nh // 2 - 1),
                        )
                    pss.append((n0, ps))
                xres = osb.tile([128, h], FP, name="xres")
                nc.sync.dma_start(out=xres, in_=x[m * 128 : (m + 1) * 128, :])
                o = osb.tile([128, h], FP, name="osum")
                for n0, ps in pss:
                    nc.vector.tensor_add(
                        out=o[:, n0 : n0 + 512], in0=ps, in1=xres[:, n0 : n0 + 512]
                    )
                if "bo" in bcast:
                    nc.vector.tensor_add(out=o, in0=o, in1=bcast["bo"])
                if stop_after == "oproj":
                    nc.sync.dma_start(out=out[m * 128 : (m + 1) * 128, :], in_=o)
                    continue
                nsub = (h + 511) // 512
                stats = lnp.tile([128, nsub, 6], FP, name="stats")
                for i in range(nsub):
                    nc.vector.bn_stats(
                        out=stats[:, i, :], in_=o[:, i * 512 : (i + 1) * 512]
                    )
                mv = lnp.tile([128, 2], FP, name="mv")
                nc.vector.bn_aggr(out=mv, in_=stats)
                std = lnp.tile([128, 1], FP, name="std")
                nc.scalar.activation(std, mv[:, 1:2], AF.Sqrt, bias=eps_sb)
                inv = lnp.tile([128, 1], FP, name="inv")
                nc.vector.reciprocal(inv, std)
                y = osb.tile([128, h], FP, name="yout")
                nc.vector.tensor_scalar(
                    out=y,
                    in0=o,
                    scalar1=mv[:, 0:1],
                    scalar2=inv,
                    op0=OP.subtract,
                    op1=OP.mult,
                )
                if "ln_gamma" in bcast:
                    nc.vector.tensor_mul(out=y, in0=y, in1=bcast["ln_gamma"])
                if "ln_beta" in bcast:
                    nc.vector.tensor_add(out=y, in0=y, in1=bcast["ln_beta"])
                nc.sync.dma_start(out=out[m * 128 : (m + 1) * 128, :], in_=y)
            if stop_after not in (None, 'oproj'):
                for m in range(qt_n):
                    dbg = osb.tile([128, h], FP, name="dbg", tag="xres")
                    if stop_after == "proj":
                        nc.vector.tensor_copy(out=dbg, in_=kt[:, 0, 0:h])
                    else:
                        nc.vector.tensor_copy(out=dbg[0:64, :sh], in_=ctx_t[0:64, 0, :])
                        nc.vector.tensor_copy(out=dbg[64:128, :sh], in_=ctx_t[64:128, 0, :])
                    nc.sync.dma_start(out=out[m * 128 : (m + 1) * 128, :], in_=dbg)

    if split:
        _split_sync_waits(nc)
    return nc


_NC_CACHE = {}


def _get_nc(s, h, nh, sh, flags):
    key = (s, h, nh, sh, tuple(sorted(flags.items())))
    if key not in _NC_CACHE:
        _NC_CACHE[key] = _build(s, h, nh, sh, flags)
    return _NC_CACHE[key]


def _prepare(hidden_states, attention_mask, Wq, bq, Wk, bk, Wv, bv, Wo, bo, ln_gamma, ln_beta):
    hs = np.ascontiguousarray(np.asarray(hidden_states, dtype=np.float32))
    b_, s_, h_ = hs.shape
    nh_ = h_ // 64
    sh_ = s_ // 2
    am = np.asarray(attention_mask, dtype=np.float32).reshape(b_, s_)
    flags = {
        "bq": bool(np.any(np.asarray(bq))),
        "bk": bool(np.any(np.asarray(bk))),
        "bv": bool(np.any(np.asarray(bv))),
        "bo": bool(np.any(np.asarray(bo))),
        "ln_gamma": not bool(np.all(np.asarray(ln_gamma) == 1.0)),
        "ln_beta": bool(np.any(np.asarray(ln_beta))),
    }
    nc = _get_nc(s_, h_, nh_, sh_, flags)

    f32c = lambda a: np.ascontiguousarray(np.asarray(a, dtype=np.float32))
    shared = {"wq": f32c(Wq), "wk": f32c(Wk), "wv": f32c(Wv), "wo": f32c(Wo)}
    for name, arr in (
        ("bq", bq),
        ("bk", bk),
        ("bv", bv),
        ("bo", bo),
        ("ln_gamma", ln_gamma),
        ("ln_beta", ln_beta),
    ):
        if flags[name]:
            shared[name] = f32c(arr)

    in_maps = []
    for c in range(N_CORES):
        bb, half = c // 2, c % 2
        mine = slice(half * sh_, (half + 1) * sh_)
        other = slice((1 - half) * sh_, (2 - half) * sh_)
        xp = np.ascontiguousarray(np.concatenate([hs[bb, mine], hs[bb, other]], axis=0))
        mp = np.ascontiguousarray(np.concatenate([am[bb, mine], am[bb, other]]))
        in_maps.append({"x": xp, "mask": mp, **shared})
    return nc, in_maps, (b_, s_, h_, sh_)


def _assemble(results, shape):
    b_, s_, h_, sh_ = shape
    out = np.empty((b_, s_, h_), dtype=np.float32)
    for c in range(N_CORES):
        bb, half = c // 2, c % 2
        out[bb, half * sh_ : (half + 1) * sh_] = results[c]["out"]
    return out


def kernel(**inputs) -> np.ndarray:
    nc, in_maps, shape = _prepare(**inputs)
    res = run_bass_kernel_spmd(nc, in_maps, core_ids=list(range(N_CORES)))
    return _assemble(res.results, shape)
